# revision 1
# baseline (speedup 1.0000x reference)
"""Ragged-sequence multi-head attention (B=16, S=1024, D=512, H=8, DH=64)
for 8 Trainium2 NeuronCores.

Strategy: data-parallel over the batch. The 16 sequences are sorted by
length; the 8 longest go to slot 0 (one per core), the 8 shortest to
slot 1. A single SPMD Bass program processes both slots with per-slot
static loop bounds equal to ceil128(max length in that slot); within a
bound, invalid key positions are masked via a per-partition additive
bias on the exp() activation, and padded query rows are zeroed via a
per-partition multiplicative mask.

Per-core pipeline (per slot, all fp32 / fp32r):
  1. x -> xT (PE transpose via identity)
  2. QT = Wq^T @ x^T, KT likewise (feature-major), V in [s, d] layout
  3. per head-pair, per q-chunk, per k-tile:
       scoresT[k, q] = K^T q   (row-packed head pair on the PE array)
       expT = exp(0.125 * scoresT + key_mask_bias)   (ACT engine)
       outT[d, q]  += V^T expT (col-packed head pair)
       denom[., q] += 1^T expT (col-packed head pair, replicated rows)
  4. outT_norm = outT * reciprocal(denom)   (DVE)
  5. out[s, d] = outT_norm^T @ Wo + bo, masked by query validity
"""

import math
import os

import numpy as np

B, S, D = 16, 1024, 512
H, DH = 8, 64
N_CORES = 8
P = 128  # partitions
KC = D // P  # 4 contraction chunks of 128
NT_MAX = S // P  # 8 key tiles max

_BUILD_CACHE: dict = {}


def _ceil128(n: int) -> int:
    return max(P, (int(n) + P - 1) // P * P)


def _build_bass(bounds: tuple[int, int]):
    """Build the Bass program for per-slot bounds (multiples of 128)."""
    from contextlib import ExitStack

    import concourse.bass as bass
    import concourse.mybir as mybir
    import concourse.tile as tile
    from concourse import bacc

    fp32 = mybir.dt.float32
    fp16 = mybir.dt.float16
    Exp = mybir.ActivationFunctionType.Exp
    mult = mybir.AluOpType.mult
    add = mybir.AluOpType.add

    nc = bacc.Bacc("TRN2", target_bir_lowering=False, debug=False)

    xin = nc.dram_tensor("xin", [2, S, D], fp32, kind="ExternalInput").ap()
    ident_d = nc.dram_tensor("ident", [P, P], fp32, kind="ExternalInput").ap()
    kbias_d = nc.dram_tensor("kbias", [2, P, NT_MAX], fp32, kind="ExternalInput").ap()
    qmask_d = nc.dram_tensor("qmask", [2, P, NT_MAX], fp32, kind="ExternalInput").ap()
    w_d = {
        name: nc.dram_tensor(name, [D, D], fp32, kind="ExternalInput").ap()
        for name in ("wq", "wk", "wv", "wo")
    }
    bo_d = nc.dram_tensor("bo", [D], fp32, kind="ExternalInput").ap()
    out_d = nc.dram_tensor("out", [2, S, D], fp32, kind="ExternalOutput").ap()

    NT = [bounds[0] // P, bounds[1] // P]
    QCH = [
        [(qs, min(512, bounds[b] - qs)) for qs in range(0, bounds[b], 512)]
        for b in (0, 1)
    ]

    with ExitStack() as ctx:
        tc = ctx.enter_context(tile.TileContext(nc))
        singles = ctx.enter_context(tc.tile_pool(name="singles", bufs=1))
        wstage_p = ctx.enter_context(tc.tile_pool(name="wstage_p", bufs=2))
        big = ctx.enter_context(tc.tile_pool(name="big", bufs=1))
        xpool = ctx.enter_context(tc.tile_pool(name="xpool", bufs=4))
        epool = ctx.enter_context(tc.tile_pool(name="epool", bufs=3))
        opool = ctx.enter_context(tc.tile_pool(name="opool", bufs=4))
        mmps = ctx.enter_context(tc.tile_pool(name="mmps", bufs=2, space="PSUM"))
        scps = ctx.enter_context(tc.tile_pool(name="scps", bufs=2, space="PSUM"))
        accps = ctx.enter_context(tc.tile_pool(name="accps", bufs=1, space="PSUM"))

        # ---- weights / constants ----
        ones64 = singles.tile([P, DH], fp16)
        nc.vector.memset(ones64, 1.0)
        w_sb = {}
        for name in ("wv", "wq", "wk", "wo"):
            w_sb[name] = singles.tile(
                [P, KC, D], fp16, name=f"w_{name}", tag=f"w_{name}"
            )
        def load_weight(name):
            ws = wstage_p.tile([P, KC, D], fp32, name=f"ws_{name}", tag="wstage")
            nc.sync.dma_start(
                out=ws, in_=w_d[name].rearrange("(kc p) n -> p kc n", p=P)
            )
            nc.scalar.copy(out=w_sb[name], in_=ws)

        for name in ("wv", "wq"):
            load_weight(name)

        # ---- phase A first: x DMAs + transposes (no weights needed) ----
        identity = singles.tile([P, P], fp32)
        nc.sync.dma_start(out=identity, in_=ident_d)
        xT = []
        for b in (0, 1):
            xT.append(big.tile([P, KC, bounds[b]], fp16, name=f"xT{b}", tag=f"xT{b}"))
            for st in range(NT[b]):
                x_tile = xpool.tile([P, D], fp32, tag="x_tile")
                nc.sync.dma_start(out=x_tile, in_=xin[b, st * P : (st + 1) * P, :])
                xt_ps = mmps.tile([P, 512], fp32, name="xt_ps", tag="mm")
                for dc in range(KC):
                    nc.tensor.transpose(
                        xt_ps[:, dc * P : (dc + 1) * P],
                        x_tile[:, dc * P : (dc + 1) * P],
                        identity,
                    )
                nc.vector.tensor_copy(
                    out=xT[b][:, :, st * P : (st + 1) * P],
                    in_=xt_ps.rearrange("p (dc c) -> p dc c", dc=KC),
                )

        for name in ("wk", "wo"):
            load_weight(name)
        kbias_sb = singles.tile([P, 2, NT_MAX], fp32)
        nc.sync.dma_start(out=kbias_sb, in_=kbias_d.rearrange("b p t -> p b t"))
        qmask_sb = singles.tile([P, 2, NT_MAX], fp32)
        nc.sync.dma_start(out=qmask_sb, in_=qmask_d.rearrange("b p t -> p b t"))
        bo_rep = singles.tile([P, D], fp32)
        bo_bcast = bass.AP(tensor=bo_d.tensor, offset=bo_d.offset, ap=[[0, P], [1, D]])
        nc.gpsimd.dma_start(out=bo_rep, in_=bo_bcast)

        # ---- V: slot 0 emitted now; slot 1 rides the filler ----
        V = [
            big.tile([P, NT[b], D], fp16, name=f"V{b}", tag=f"V{b}")
            for b in (0, 1)
        ]

        def v_units(b, st):
            ps_box = []

            def mk_mm(kc):
                def emit():
                    if not ps_box:
                        ps_box.append(
                            mmps.tile([P, 512], fp32, name="v_ps", tag="mm")
                        )
                    nc.tensor.matmul(
                        ps_box[0],
                        xT[b][:, kc, st * P : (st + 1) * P],
                        w_sb["wv"][:, kc, :],
                        start=(kc == 0),
                        stop=(kc == KC - 1),
                    )
                return emit

            def fin():
                nc.vector.tensor_copy(out=V[b][:, st, :], in_=ps_box[0])

            return [mk_mm(kc) for kc in range(KC)] + [fin]

        for st in range(NT[0]):
            for u in v_units(0, st):
                u()

        QT = [
            big.tile([P, KC, bounds[b]], fp16, name=f"QT{b}", tag=f"QT{b}")
            for b in (0, 1)
        ]
        KT = [
            big.tile([P, KC, bounds[b]], fp16, name=f"KT{b}", tag=f"KT{b}")
            for b in (0, 1)
        ]
        outT = [
            big.tile([P, KC, bounds[b]], fp16, name=f"oT{b}", tag=f"oT{b}")
            for b in (0, 1)
        ]

        def qtkt_units(b, hp, dst, wname, qs, w):
            ps_box = []

            def mk_mm(kc):
                def emit():
                    if not ps_box:
                        ps_box.append(
                            mmps.tile([P, 512], fp32, name="qk_ps", tag="mm")
                        )
                    nc.tensor.matmul(
                        ps_box[0][:, :w],
                        w_sb[wname][:, kc, hp * P : (hp + 1) * P],
                        xT[b][:, kc, qs : qs + w],
                        start=(kc == 0),
                        stop=(kc == KC - 1),
                    )
                return emit

            def fin():
                nc.vector.tensor_copy(
                    out=dst[:, hp, qs : qs + w], in_=ps_box[0][:, :w]
                )

            return [mk_mm(kc) for kc in range(KC)] + [fin]

        def outproj_units(b, st):
            ps_box = []

            def mk_mm(hc):
                def emit():
                    if not ps_box:
                        ps_box.append(
                            mmps.tile([P, 512], fp32, name="fo_ps", tag="mm")
                        )
                    nc.tensor.matmul(
                        ps_box[0],
                        outT[b][:, hc, st * P : (st + 1) * P],
                        w_sb["wo"][:, hc, :],
                        start=(hc == 0),
                        stop=(hc == KC - 1),
                    )
                return emit

            def fin():
                fout = opool.tile([P, D], fp32, tag="fout")
                nc.vector.tensor_tensor(fout, ps_box[0], bo_rep, add)
                nc.vector.tensor_scalar_mul(
                    fout, fout, qmask_sb[:, b, st : st + 1]
                )
                nc.sync.dma_start(
                    out=out_d[b, st * P : (st + 1) * P, :], in_=fout
                )

            return [mk_mm(hc) for hc in range(KC)] + [fin]

        def attn_chunk(b, hp, qs, w, filler, iters_left):
            o_ps = accps.tile([P, 512], fp32, name="o_ps", tag="o_ps")
            d_ps = accps.tile([P, 512], fp32, name="d_ps", tag="d_ps")
            nt = NT[b]

            def emit_scores_exp(kt):
                s_pair = scps.tile([P, 1024], fp32, name="s_pair", tag="s_pair")
                nc.tensor.matmul(
                    s_pair[:, 0:w],
                    KT[b][0:DH, hp, kt * P : (kt + 1) * P],
                    QT[b][0:DH, hp, qs : qs + w],
                    start=True, stop=True, tile_position=(0, 0),
                )
                nc.tensor.matmul(
                    s_pair[:, 512 : 512 + w],
                    KT[b][DH:P, hp, kt * P : (kt + 1) * P],
                    QT[b][DH:P, hp, qs : qs + w],
                    start=True, stop=True, tile_position=(DH, 0),
                )
                e_pair = epool.tile([P, 2, 512], fp16, name="e_pair", tag="e_pair")
                nc.scalar.activation(
                    e_pair[:, :, :w],
                    s_pair.rearrange("p (h q) -> p h q", h=2)[:, :, :w],
                    Exp, bias=kbias_sb[:, b, kt : kt + 1], scale=DH**-0.5,
                )
                return e_pair

            def emit_pv(kt, e_pair):
                first, last = kt == 0, kt == nt - 1
                nc.tensor.matmul(
                    o_ps[0:DH, :w], V[b][:, kt, hp * P : hp * P + DH],
                    e_pair[:, 0, :w], start=first, stop=last,
                    tile_position=(0, 0), skip_group_check=True,
                )
                nc.tensor.matmul(
                    o_ps[DH:P, :w], V[b][:, kt, hp * P + DH : (hp + 1) * P],
                    e_pair[:, 1, :w], start=first, stop=last,
                    tile_position=(0, DH), skip_group_check=True,
                )
                nc.tensor.matmul(
                    d_ps[0:DH, :w], ones64, e_pair[:, 0, :w],
                    start=first, stop=last,
                    tile_position=(0, 0), skip_group_check=True,
                )
                nc.tensor.matmul(
                    d_ps[DH:P, :w], ones64, e_pair[:, 1, :w],
                    start=first, stop=last,
                    tile_position=(0, DH), skip_group_check=True,
                )

            pending = None
            for kt in range(nt):
                e_pair = emit_scores_exp(kt)
                if pending is not None:
                    emit_pv(*pending)
                pending = (kt, e_pair)
                if filler and iters_left[0] > 0:
                    k = -(-len(filler) // iters_left[0])
                    for _ in range(min(k, len(filler))):
                        filler.pop(0)()
                iters_left[0] -= 1
            emit_pv(*pending)
            rrep = epool.tile([P, 512], fp32, tag="rrep", bufs=2)
            nc.vector.reciprocal_approx_fast(out=rrep[:, :w], in_=d_ps[:, :w])
            nc.vector.tensor_tensor(
                outT[b][:, hp, qs : qs + w], o_ps[:, :w], rrep[:, :w], mult
            )

        # ---- choreographed emission ----
        for dst, wname in ((QT[0], "wq"), (KT[0], "wk")):
            for qs, w in QCH[0]:
                for u in qtkt_units(0, 0, dst, wname, qs, w):
                    u()

        blocks = [(0, hp) for hp in range(KC)] + [(1, hp) for hp in range(KC)]
        during_block = [[] for _ in blocks]
        # V for slot 1 drains during slot0 hp0/hp1
        for st in range(NT[1]):
            during_block[st % 2].extend(v_units(1, st))
        for j in range(1, len(blocks)):
            b, hp = blocks[j]
            for dst, wname in ((QT[b], "wq"), (KT[b], "wk")):
                for qs, w in QCH[b]:
                    during_block[j - 1].extend(
                        qtkt_units(b, hp, dst, wname, qs, w)
                    )
        # slot-0 output projection rides along slot-1's attention blocks
        s1_blocks = list(range(KC, 2 * KC))
        d0_units = [u for st in range(NT[0]) for u in outproj_units(0, st)]
        per_block = -(-len(d0_units) // len(s1_blocks))
        for i, j in enumerate(s1_blocks):
            during_block[j].extend(d0_units[i * per_block : (i + 1) * per_block])

        filler: list = []
        for i, (b, hp) in enumerate(blocks):
            filler.extend(during_block[i])
            iters_left = [len(QCH[b]) * NT[b]]
            for qs, w in QCH[b]:
                attn_chunk(b, hp, qs, w, filler, iters_left)
            while filler:
                filler.pop(0)()

        # slot-1 output projection (tail)
        for st in range(NT[1]):
            for u in outproj_units(1, st):
                u()

    nc.compile()
    return nc


def _get_program(bounds: tuple[int, int]):
    key = bounds
    if key not in _BUILD_CACHE:
        _BUILD_CACHE[key] = _build_bass(bounds)
    return _BUILD_CACHE[key]


def kernel(x, seq_lens, Wq, Wk, Wv, Wo, bo) -> np.ndarray:
    from concourse.bass_utils import run_bass_kernel_spmd

    x = np.ascontiguousarray(np.asarray(x, dtype=np.float32))
    seq_lens_np = np.asarray(seq_lens, dtype=np.int32)
    Wq = np.ascontiguousarray(np.asarray(Wq, dtype=np.float32))
    Wk = np.ascontiguousarray(np.asarray(Wk, dtype=np.float32))
    Wv = np.ascontiguousarray(np.asarray(Wv, dtype=np.float32))
    Wo = np.ascontiguousarray(np.asarray(Wo, dtype=np.float32))
    bo = np.ascontiguousarray(np.asarray(bo, dtype=np.float32))

    # Sort sequences by length: longest 8 -> slot 0, rest -> slot 1.
    order = np.argsort(-seq_lens_np, kind="stable")
    slot_seqs = [order[:N_CORES], order[N_CORES:]]
    bounds = tuple(int(_ceil128(seq_lens_np[s].max())) for s in slot_seqs)

    nc = _get_program(bounds)

    # Per-partition masks laid out as [slot, p, tile]: position t*128+p.
    pos = (np.arange(NT_MAX)[None, :] * P + np.arange(P)[:, None]).astype(np.int32)
    in_maps = []
    for c in range(N_CORES):
        seq_pair = [int(slot_seqs[0][c]), int(slot_seqs[1][c])]
        xin = np.stack([x[seq_pair[0]], x[seq_pair[1]]])
        kbias = np.zeros((2, P, NT_MAX), dtype=np.float32)
        qmask = np.zeros((2, P, NT_MAX), dtype=np.float32)
        for slot, seq in enumerate(seq_pair):
            valid = pos < int(seq_lens_np[seq])
            kbias[slot] = np.where(valid, 0.0, -60.0)
            qmask[slot] = valid.astype(np.float32)
        in_maps.append(
            {
                "xin": xin,
                "ident": np.eye(P, dtype=np.float32),
                "kbias": kbias,
                "qmask": qmask,
                "wq": Wq,
                "wk": Wk,
                "wv": Wv,
                "wo": Wo,
                "bo": bo,
            }
        )

    trace = bool(int(os.environ.get("KERNEL_TRACE", "0")))
    res = run_bass_kernel_spmd(
        nc, in_maps, core_ids=list(range(N_CORES)), trace=trace
    )
    kernel.last_results = res

    out = np.zeros((B, S, D), dtype=np.float32)
    for c in range(N_CORES):
        out[int(slot_seqs[0][c])] = res.results[c]["out"][0]
        out[int(slot_seqs[1][c])] = res.results[c]["out"][1]
    return out



# revision 7
# speedup vs baseline: 1.0070x; 1.0070x over previous
"""Ragged-sequence multi-head attention (B=16, S=1024, D=512, H=8, DH=64)
for 8 Trainium2 NeuronCores.

Strategy: data-parallel SPMD over a windowed ragged structure. The host
splits the 16 sequences into vseqs (contiguous q-tile ranges) and packs
them onto 8 cores; the shared program is parametrized by K-WINDOWS
(static k-tile ranges holding one parent sequence's K/V per core) and
Q-CHUNKS (static q-tile ranges bound to one window with a static k-cap).
Each core's in_map places its own parents' tokens into the static
layout, so all cores run one instruction stream over different data.

Masking: invalid key rows have zeroed x (V rows = 0, scores = 0 ->
exp = 1) and zeroed "validity" columns in the fused V|valid layout, so
both the numerator and the denominator are exact without any exp bias.

Mixed precision: Q/K projections fp16 -> QT/KT stored fp8e4 ->
DoubleRow fp8 scores (2x); exp on ACT (fp16 out); PV+denominator fused
as one fp16 [128,128] matmul per head (V dims | validity columns);
fp16 V/out projections.
"""

import math
import os

import numpy as np
import ml_dtypes

B, S, D = 16, 1024, 512
H, DH = 8, 64
N_CORES = 8
P = 128
KC = D // P  # 4

FP8 = ml_dtypes.float8_e4m3fn

_BUILD_CACHE: dict = {}


# --------------------------------------------------------------------------
# structure solver
# --------------------------------------------------------------------------

def _solve_structure(seq_lens):
    """Returns (windows, chunks, assign)."""
    import random

    nk = [max(1, math.ceil(int(l) / P)) for l in seq_lens]
    total = sum(k * k for k in nk)

    def mk_vseqs(T):
        vseqs = []
        for i, k in enumerate(nk):
            parts = max(1, math.ceil(k * k / T))
            base, rem, qt = k // parts, k - (k // parts) * parts, 0
            for p_ in range(parts):
                n = base + (1 if p_ < rem else 0)
                if n:
                    vseqs.append((i, qt, n, k))
                    qt += n
        return vseqs

    def cost_of(bins):
        M = max(len(b) for b in bins)
        Csum = NKT = NQ = 0
        for m in range(M):
            col = [sorted(b, key=lambda v: -v[3])[m] if len(b) > m else None
                   for b in bins]
            NKT += max((v[3] if v else 0) for v in col)
            counts = [v[2] if v else 0 for v in col]
            nks = [v[3] if v else 0 for v in col]
            caps = [max((nks[c] if i < counts[c] else 0) for c in range(8))
                    for i in range(max(counts))]
            Csum += sum(caps)
            NQ += len(caps)
        return 1536 * Csum + 3072 * NKT + 2560 * NQ

    best_bins, best_obj = None, None
    for t_mult in (0.85, 1.0, 1.2):
        T = max(4, math.ceil(total / 8 * t_mult))
        vs = sorted(mk_vseqs(T), key=lambda v: (-v[3], -v[2]))
        bins = [[] for _ in range(8)]
        loads = [0] * 8
        for v in vs:
            b = min(range(8), key=lambda j: loads[j])
            bins[b].append(v)
            loads[b] += v[2] * v[3]
        rng = random.Random(0)
        cur = cost_of(bins)
        for _ in range(3000):
            b1, b2 = rng.randrange(8), rng.randrange(8)
            if b1 == b2 or not bins[b1]:
                continue
            i1 = rng.randrange(len(bins[b1]))
            v1 = bins[b1][i1]
            if rng.random() < 0.5 and bins[b2]:
                i2 = rng.randrange(len(bins[b2]))
                v2 = bins[b2][i2]
                bins[b1][i1], bins[b2][i2] = v2, v1
                o = cost_of(bins)
                if o <= cur:
                    cur = o
                else:
                    bins[b1][i1], bins[b2][i2] = v1, v2
            else:
                bins[b1].pop(i1)
                bins[b2].append(v1)
                o = cost_of(bins)
                if o <= cur:
                    cur = o
                else:
                    bins[b2].pop()
                    bins[b1].insert(i1, v1)
        if best_obj is None or cur < best_obj:
            best_obj, best_bins = cur, [list(b) for b in bins]

    bins = [sorted(b, key=lambda v: -v[3]) for b in best_bins]
    M = max(len(b) for b in bins)
    windows = []
    chunks = []
    assign = [[None] * M for _ in range(8)]
    qoff = 0
    for m in range(M):
        col = [b[m] if len(b) > m else None for b in bins]
        windows.append(max((v[3] if v else 0) for v in col))
        for c in range(8):
            if col[c] is not None:
                assign[c][m] = (col[c][0], col[c][1], col[c][2])
        counts = [v[2] if v else 0 for v in col]
        nks = [v[3] if v else 0 for v in col]
        caps = [max((nks[c] if i < counts[c] else 0) for c in range(8))
                for i in range(max(counts))]
        i = 0
        while i < len(caps):
            jx = i
            while jx < len(caps) and caps[jx] == caps[i] and jx - i < 4:
                jx += 1
            chunks.append((qoff + i, jx - i, m, caps[i]))
            i = jx
        qoff += len(caps)
    return tuple(windows), tuple(chunks), assign


# --------------------------------------------------------------------------
# bass program
# --------------------------------------------------------------------------

def _build_bass(windows, chunks, debug=False):
    from contextlib import ExitStack

    import concourse.bass as bass
    import concourse.mybir as mybir
    import concourse.tile as tile
    from concourse import bacc

    fp32 = mybir.dt.float32
    fp16 = mybir.dt.float16
    fp8 = mybir.dt.float8e4
    Exp = mybir.ActivationFunctionType.Exp
    DR = mybir.MatmulPerfMode.DoubleRow
    mult = mybir.AluOpType.mult
    add = mybir.AluOpType.add

    NKT = sum(windows)
    NQ = sum(c[1] for c in chunks)
    NTOK_K = NKT * P
    NTOK_Q = NQ * P
    woff = [0]
    for w_ in windows:
        woff.append(woff[-1] + w_)

    nc = bacc.Bacc("TRN2", target_bir_lowering=False, debug=False)

    xq16_d = nc.dram_tensor("xq16", [P, KC, NTOK_Q], fp16, kind="ExternalInput").ap()
    xk16_d = nc.dram_tensor("xk16", [P, KC, NTOK_K], fp16, kind="ExternalInput").ap()
    wq16_d = nc.dram_tensor("wq16", [P, 2, 2, KC, P], fp16, kind="ExternalInput").ap()
    wk16_d = nc.dram_tensor("wk16", [P, 2, 2, KC, P], fp16, kind="ExternalInput").ap()
    wv16_d = nc.dram_tensor("wv16", [P, KC, D], fp16, kind="ExternalInput").ap()
    wo16_d = nc.dram_tensor("wo16", [P, KC, D], fp16, kind="ExternalInput").ap()
    vones_d = nc.dram_tensor("vones", [P, NKT, H, DH], fp16, kind="ExternalInput").ap()
    bo_d = nc.dram_tensor("bo", [D], fp32, kind="ExternalInput").ap()
    out_d = nc.dram_tensor("out", [NTOK_Q, D], fp16, kind="ExternalOutput").ap()
    if debug:
        dbg_qt = nc.dram_tensor("dbg_qt", [P, 2, 2, NTOK_Q], fp8, kind="ExternalOutput").ap()
        dbg_kt = nc.dram_tensor("dbg_kt", [P, 2, 2, NTOK_K], fp8, kind="ExternalOutput").ap()
        dbg_v = nc.dram_tensor("dbg_v", [P, NKT, H, P], fp16, kind="ExternalOutput").ap()
        dbg_ot = nc.dram_tensor("dbg_ot", [P, KC, NTOK_Q], fp16, kind="ExternalOutput").ap()

    with ExitStack() as ctx:
        tc = ctx.enter_context(tile.TileContext(nc))
        singles = ctx.enter_context(tc.tile_pool(name="singles", bufs=1))
        fpool = ctx.enter_context(tc.tile_pool(name="fpool", bufs=3))
        epool = ctx.enter_context(tc.tile_pool(name="epool", bufs=3))
        rpool = ctx.enter_context(tc.tile_pool(name="rpool", bufs=2))
        mmps = ctx.enter_context(tc.tile_pool(name="mmps", bufs=2, space="PSUM"))
        scps = ctx.enter_context(tc.tile_pool(name="scps", bufs=2, space="PSUM"))
        accps = ctx.enter_context(tc.tile_pool(name="accps", bufs=1, space="PSUM"))

        # ---- static inputs ----
        wq16 = singles.tile([P, 2, 2, KC, P], fp16)
        nc.sync.dma_start(out=wq16, in_=wq16_d)
        wk16 = singles.tile([P, 2, 2, KC, P], fp16)
        nc.sync.dma_start(out=wk16, in_=wk16_d)
        wv16 = singles.tile([P, KC, D], fp16)
        nc.sync.dma_start(out=wv16, in_=wv16_d)
        wo16 = singles.tile([P, KC, D], fp16)
        nc.sync.dma_start(out=wo16, in_=wo16_d)
        xk16 = singles.tile([P, KC, NTOK_K], fp16)
        nc.sync.dma_start(out=xk16, in_=xk16_d)
        xq16 = singles.tile([P, KC, NTOK_Q], fp16)
        nc.sync.dma_start(out=xq16, in_=xq16_d)
        bo_rep = singles.tile([P, D], fp32)
        bo_bcast = bass.AP(tensor=bo_d.tensor, offset=bo_d.offset,
                           ap=[[0, P], [1, D]])
        nc.gpsimd.dma_start(out=bo_rep, in_=bo_bcast)

        KT8 = singles.tile([P, 2, 2, NTOK_K], fp8, name="KT8")
        QT8 = singles.tile([P, 2, 2, NTOK_Q], fp8, name="QT8")
        V128 = singles.tile([P, NKT, H, P], fp16, name="V128")
        outT = singles.tile([P, KC, NTOK_Q], fp16, name="outT")

        # validity columns of V at 0:DH (denominator lands in PSUM rows
        # 0:64 because the custom-DVE reciprocal drops input partition
        # offsets); V dims at DH:P
        nc.sync.dma_start(out=V128[:, :, :, 0:DH], in_=vones_d)

        # ---- projection unit emitters ----
        def qk_proj(dst, w16, x16, qs, w):
            for jj in range(2):
                for ii in range(2):
                    ps = mmps.tile([P, 512], fp32, name="qk_ps", tag="mm")
                    for kc in range(KC):
                        nc.tensor.matmul(
                            ps[:, :w],
                            w16[:, jj, ii, kc, :],
                            x16[:, kc, qs : qs + w],
                            start=(kc == 0), stop=(kc == KC - 1),
                        )
                    nc.vector.tensor_copy(
                        out=dst[:, jj, ii, qs : qs + w], in_=ps[:, :w]
                    )

        def v_proj(kt):
            ps = mmps.tile([P, 512], fp32, name="v_ps", tag="mm")
            for kc in range(KC):
                nc.tensor.matmul(
                    ps,
                    xk16[:, kc, kt * P : (kt + 1) * P],
                    wv16[:, kc, :],
                    start=(kc == 0), stop=(kc == KC - 1),
                )
            nc.vector.tensor_copy(
                out=V128[:, kt, :, DH:P],
                in_=ps.rearrange("p (h d) -> p h d", h=H),
            )

        def o_proj(qt):
            ps = mmps.tile([P, 512], fp32, name="o_ps", tag="mm")
            for g in range(KC):
                nc.tensor.matmul(
                    ps,
                    outT[:, g, qt * P : (qt + 1) * P],
                    wo16[:, g, :],
                    start=(g == 0), stop=(g == KC - 1),
                )
            fout = fpool.tile([P, D], fp16, tag="fout")
            nc.vector.tensor_tensor(fout, ps, bo_rep, add)
            nc.sync.dma_start(out=out_d[qt * P : (qt + 1) * P, :], in_=fout)

        # ---- emission ----
        for qs in range(0, NTOK_K, 512):
            qk_proj(KT8, wk16, xk16, qs, min(512, NTOK_K - qs))
        for qs in range(0, NTOK_Q, 512):
            qk_proj(QT8, wq16, xq16, qs, min(512, NTOK_Q - qs))

        v_done = [False] * len(windows)

        def ensure_v(m):
            if not v_done[m]:
                for kt in range(woff[m], woff[m] + windows[m]):
                    v_proj(kt)
                v_done[m] = True

        def attn_chunk(qt_off, ntiles, m, cap):
            qs, w = qt_off * P, ntiles * P
            # narrow chunks batch k-tiles into one PSUM tile / exp call;
            # flat [P, 1024] tiles: sub-block b covers (kt-in-batch, head j)
            ktc = max(1, 512 // w)  # k-tiles per scores tile
            for hl in range(4):
                o_pd = accps.tile([P, 2, 512], fp32, name="o_pd", tag="o_pd")
                # sub-block stride: a matmul output must stay inside one
                # 512-float PSUM bank, so 384-wide blocks get stride 512
                bs = 512 if w == 384 else w
                for kt0 in range(0, cap, ktc):
                    kn = min(ktc, cap - kt0)
                    s_t = scps.tile([P, 1024], fp32, name="s_t", tag="s_t")
                    e_t = epool.tile([P, 1024], fp16, name="e_t", tag="e_t")
                    for dk in range(kn):
                        ktg = woff[m] + kt0 + dk
                        for jj in range(2):
                            off = (dk * 2 + jj) * bs
                            nc.tensor.matmul(
                                s_t[:, off : off + w],
                                KT8[32 * hl : 32 * hl + 32, jj, :,
                                    ktg * P : (ktg + 1) * P],
                                QT8[32 * hl : 32 * hl + 32, jj, :, qs : qs + w],
                                start=True, stop=True, perf_mode=DR,
                                tile_position=(32 * hl, 0),
                            )
                    if bs == w:
                        nc.scalar.activation(
                            e_t[:, 0 : kn * 2 * w], s_t[:, 0 : kn * 2 * w],
                            Exp, scale=0.125,
                        )
                    else:
                        nc.scalar.activation(
                            e_t.rearrange("p (j q) -> p j q", j=2)[:, :, :w],
                            s_t.rearrange("p (j q) -> p j q", j=2)[:, :, :w],
                            Exp, scale=0.125,
                        )
                    for dk in range(kn):
                        ktg = woff[m] + kt0 + dk
                        kt = kt0 + dk
                        for jj in range(2):
                            off = (dk * 2 + jj) * bs
                            nc.tensor.matmul(
                                o_pd[:, jj, :w],
                                V128[:, ktg, hl + 4 * jj, :],
                                e_t[:, off : off + w],
                                start=(kt == 0), stop=(kt == cap - 1),
                                skip_group_check=True,
                            )
                rrep = rpool.tile([DH, 2, 512], fp32, tag="rrep")
                nc.vector.reciprocal_approx_fast(
                    out=rrep[:, :, :w], in_=o_pd[0:DH, :, :w]
                )
                for jj in range(2):
                    nc.vector.tensor_tensor(
                        outT[DH * jj : DH * jj + DH, hl, qs : qs + w],
                        o_pd[DH:P, jj, :w],
                        rrep[:, jj, :w],
                        mult,
                    )

        for (qt_off, ntiles, m, cap) in chunks:
            ensure_v(m)
            attn_chunk(qt_off, ntiles, m, cap)
            for qt in range(qt_off, qt_off + ntiles):
                o_proj(qt)

        if debug:
            nc.sync.dma_start(out=dbg_qt, in_=QT8)
            nc.sync.dma_start(out=dbg_kt, in_=KT8)
            nc.sync.dma_start(out=dbg_v, in_=V128)
            nc.sync.dma_start(out=dbg_ot, in_=outT)

    nc.compile()
    return nc


def _get_program(windows, chunks, debug):
    key = (windows, chunks, debug)
    if key not in _BUILD_CACHE:
        _BUILD_CACHE[key] = _build_bass(windows, chunks, debug)
    return _BUILD_CACHE[key]


# --------------------------------------------------------------------------
# host glue
# --------------------------------------------------------------------------

def _xt(tokens_x):
    """[T, D] fp32 -> [P, KC, T] transposed layout."""
    t = tokens_x.T.reshape(KC, P, tokens_x.shape[0]).transpose(1, 0, 2)
    return np.ascontiguousarray(t)


def _first_qt(chunks, m):
    for (qt_off, ntiles, mm, cap) in chunks:
        if mm == m:
            return qt_off
    raise ValueError(m)


def kernel(x, seq_lens, Wq, Wk, Wv, Wo, bo) -> np.ndarray:
    from concourse.bass_utils import run_bass_kernel_spmd

    x = np.asarray(x, dtype=np.float32)
    seq_lens_np = np.asarray(seq_lens, dtype=np.int32)
    Wq = np.asarray(Wq, dtype=np.float32)
    Wk = np.asarray(Wk, dtype=np.float32)
    Wv = np.asarray(Wv, dtype=np.float32)
    Wo = np.asarray(Wo, dtype=np.float32)
    bo = np.asarray(bo, dtype=np.float32)

    windows, chunks, assign = _solve_structure(seq_lens_np)
    NKT = sum(windows)
    NQ = sum(c[1] for c in chunks)
    woff = [0]
    for w_ in windows:
        woff.append(woff[-1] + w_)

    debug = bool(int(os.environ.get("KERNEL_DEBUG", "0")))
    nc = _get_program(windows, chunks, debug)

    # weight pre-arrangement (shared across cores)
    pidx = np.arange(P)
    hl_of = pidx // 32
    dlow = pidx % 32
    col = np.zeros((2, 2, P), dtype=np.int64)
    for jj in range(2):
        for ii in range(2):
            col[jj, ii] = 64 * (hl_of + 4 * jj) + dlow + 32 * ii

    def arrange_qk(W):
        # [c(128), j, i, kc, m] = W[kc*128+c, col(m,j,i)]
        a = np.zeros((P, 2, 2, KC, P), dtype=np.float32)
        for kc in range(KC):
            rows = np.arange(P) + kc * 128
            for jj in range(2):
                for ii in range(2):
                    a[:, jj, ii, kc, :] = W[rows[:, None], col[jj, ii][None, :]]
        return a.astype(np.float16)

    wq16 = arrange_qk(Wq)
    wk16 = arrange_qk(Wk)
    wv16 = np.ascontiguousarray(
        Wv.reshape(KC, P, D).transpose(1, 0, 2)
    ).astype(np.float16)
    wo16 = np.zeros((P, KC, D), dtype=np.float32)
    for g in range(KC):
        rows = 64 * (g + 4 * (pidx // 64)) + pidx % 64
        wo16[:, g, :] = Wo[rows, :]
    wo16 = wo16.astype(np.float16)

    in_maps = []
    for c in range(N_CORES):
        xk = np.zeros((NKT * P, D), dtype=np.float32)
        vones = np.zeros((P, NKT, H, DH), dtype=np.float16)
        for m, a in enumerate(assign[c]):
            if a is None:
                continue
            seq = a[0]
            L = int(seq_lens_np[seq])
            nkt_par = min(math.ceil(L / P), windows[m])
            n = min(L, nkt_par * P)
            xk[woff[m] * P : woff[m] * P + n] = x[seq, :n]
            pos = (np.arange(windows[m])[None, :] * P
                   + np.arange(P)[:, None])
            valid = (pos < L).astype(np.float16)  # [P, win]
            vones[:, woff[m] : woff[m] + windows[m], :, :] = (
                valid[:, :, None, None]
            )
        xq = np.zeros((NQ * P, D), dtype=np.float32)
        for (qt_off, ntiles, m, cap) in chunks:
            a = assign[c][m]
            if a is None:
                continue
            seq, qt0, njobs = a
            L = int(seq_lens_np[seq])
            for idx in range(ntiles):
                gidx = qt_off + idx
                job = gidx - _first_qt(chunks, m)
                if job < njobs:
                    r0 = (qt0 + job) * P
                    n = max(0, min(L - r0, P))
                    if n > 0:
                        xq[gidx * P : gidx * P + n] = x[seq, r0 : r0 + n]
        in_maps.append({
            "xq16": _xt(xq).astype(np.float16),
            "xk16": _xt(xk).astype(np.float16),
            "wq16": wq16, "wk16": wk16, "wv16": wv16, "wo16": wo16,
            "vones": vones, "bo": bo,
        })

    trace = bool(int(os.environ.get("KERNEL_TRACE", "0")))
    res = run_bass_kernel_spmd(
        nc, in_maps, core_ids=list(range(N_CORES)), trace=trace
    )
    kernel.last_results = res

    out = np.zeros((B, S, D), dtype=np.float32)
    for c in range(N_CORES):
        o = res.results[c]["out"].astype(np.float32)
        for (qt_off, ntiles, m, cap) in chunks:
            a = assign[c][m]
            if a is None:
                continue
            seq, qt0, njobs = a
            L = int(seq_lens_np[seq])
            for idx in range(ntiles):
                gidx = qt_off + idx
                job = gidx - _first_qt(chunks, m)
                if job < njobs:
                    r0 = (qt0 + job) * P
                    n = max(0, min(L - r0, P))
                    if n > 0:
                        out[seq, r0 : r0 + n] = o[gidx * P : gidx * P + n]
    return out


# revision 10
# speedup vs baseline: 1.1260x; 1.1182x over previous
"""Ragged-sequence multi-head attention (B=16, S=1024, D=512, H=8, DH=64)
for 8 Trainium2 NeuronCores.

Strategy: data-parallel SPMD over a windowed ragged structure. The host
splits the 16 sequences into vseqs (contiguous q-tile ranges) and packs
them onto 8 cores; the shared program is parametrized by K-WINDOWS
(static k-tile ranges holding one parent sequence's K/V per core) and
Q-CHUNKS (static q-tile ranges bound to one window with a static k-cap).
Each core's in_map places its own parents' tokens into the static
layout, so all cores run one instruction stream over different data.

Masking: invalid key rows have zeroed x (V rows = 0, scores = 0 ->
exp = 1) and zeroed "validity" columns in the fused V|valid layout, so
both the numerator and the denominator are exact without any exp bias.

Mixed precision: Q/K projections fp16 -> QT/KT stored fp8e4 ->
DoubleRow fp8 scores (2x); exp on ACT (fp16 out); PV+denominator fused
as one fp16 [128,128] matmul per head (V dims | validity columns);
fp16 V/out projections.
"""

import math
import os

import numpy as np
import ml_dtypes

B, S, D = 16, 1024, 512
H, DH = 8, 64
N_CORES = 8
P = 128
KC = D // P  # 4

FP8 = ml_dtypes.float8_e4m3fn

_BUILD_CACHE: dict = {}


# --------------------------------------------------------------------------
# structure solver
# --------------------------------------------------------------------------

def _solve_structure(seq_lens):
    """Returns (windows, chunks, assign)."""
    import random

    nk = [max(1, math.ceil(int(l) / P)) for l in seq_lens]
    total = sum(k * k for k in nk)

    def mk_vseqs(T):
        vseqs = []
        for i, k in enumerate(nk):
            parts = max(1, math.ceil(k * k / T))
            base, rem, qt = k // parts, k - (k // parts) * parts, 0
            for p_ in range(parts):
                n = base + (1 if p_ < rem else 0)
                if n:
                    vseqs.append((i, qt, n, k))
                    qt += n
        return vseqs

    def cost_of(bins):
        M = max(len(b) for b in bins)
        Csum = NKT = NQ = 0
        for m in range(M):
            col = [sorted(b, key=lambda v: -v[3])[m] if len(b) > m else None
                   for b in bins]
            NKT += max((v[3] if v else 0) for v in col)
            counts = [v[2] if v else 0 for v in col]
            nks = [v[3] if v else 0 for v in col]
            caps = [max((nks[c] if i < counts[c] else 0) for c in range(8))
                    for i in range(max(counts))]
            Csum += sum(caps)
            NQ += len(caps)
        return 1536 * Csum + 3072 * NKT + 2560 * NQ

    best_bins, best_obj = None, None
    for t_mult in (0.85, 1.0, 1.2):
        T = max(4, math.ceil(total / 8 * t_mult))
        vs = sorted(mk_vseqs(T), key=lambda v: (-v[3], -v[2]))
        bins = [[] for _ in range(8)]
        loads = [0] * 8
        for v in vs:
            b = min(range(8), key=lambda j: loads[j])
            bins[b].append(v)
            loads[b] += v[2] * v[3]
        rng = random.Random(0)
        cur = cost_of(bins)
        for _ in range(3000):
            b1, b2 = rng.randrange(8), rng.randrange(8)
            if b1 == b2 or not bins[b1]:
                continue
            i1 = rng.randrange(len(bins[b1]))
            v1 = bins[b1][i1]
            if rng.random() < 0.5 and bins[b2]:
                i2 = rng.randrange(len(bins[b2]))
                v2 = bins[b2][i2]
                bins[b1][i1], bins[b2][i2] = v2, v1
                o = cost_of(bins)
                if o <= cur:
                    cur = o
                else:
                    bins[b1][i1], bins[b2][i2] = v1, v2
            else:
                bins[b1].pop(i1)
                bins[b2].append(v1)
                o = cost_of(bins)
                if o <= cur:
                    cur = o
                else:
                    bins[b2].pop()
                    bins[b1].insert(i1, v1)
        if best_obj is None or cur < best_obj:
            best_obj, best_bins = cur, [list(b) for b in bins]

    bins = [sorted(b, key=lambda v: -v[3]) for b in best_bins]
    M = max(len(b) for b in bins)
    windows = []
    chunks = []
    assign = [[None] * M for _ in range(8)]
    qoff = 0
    for m in range(M):
        col = [b[m] if len(b) > m else None for b in bins]
        windows.append(max((v[3] if v else 0) for v in col))
        for c in range(8):
            if col[c] is not None:
                assign[c][m] = (col[c][0], col[c][1], col[c][2])
        counts = [v[2] if v else 0 for v in col]
        nks = [v[3] if v else 0 for v in col]
        caps = [max((nks[c] if i < counts[c] else 0) for c in range(8))
                for i in range(max(counts))]
        i = 0
        while i < len(caps):
            jx = i
            while jx < len(caps) and caps[jx] == caps[i] and jx - i < 4:
                jx += 1
            chunks.append((qoff + i, jx - i, m, caps[i]))
            i = jx
        qoff += len(caps)
    return tuple(windows), tuple(chunks), assign


# --------------------------------------------------------------------------
# bass program
# --------------------------------------------------------------------------

def _build_bass(windows, chunks, debug=False):
    from contextlib import ExitStack

    import concourse.bass as bass
    import concourse.mybir as mybir
    import concourse.tile as tile
    from concourse import bacc

    fp32 = mybir.dt.float32
    fp16 = mybir.dt.float16
    fp8 = mybir.dt.float8e4
    Exp = mybir.ActivationFunctionType.Exp
    DR = mybir.MatmulPerfMode.DoubleRow
    mult = mybir.AluOpType.mult
    add = mybir.AluOpType.add

    NKT = sum(windows)
    NQ = sum(c[1] for c in chunks)
    NTOK_K = NKT * P
    NTOK_Q = NQ * P
    woff = [0]
    for w_ in windows:
        woff.append(woff[-1] + w_)

    nc = bacc.Bacc("TRN2", target_bir_lowering=False, debug=False)

    xq16_d = nc.dram_tensor("xq16", [P, KC, NTOK_Q], fp16, kind="ExternalInput").ap()
    xk16_d = nc.dram_tensor("xk16", [P, KC, NTOK_K], fp16, kind="ExternalInput").ap()
    wq16_d = nc.dram_tensor("wq16", [P, 2, 2, KC, P], fp16, kind="ExternalInput").ap()
    wk16_d = nc.dram_tensor("wk16", [P, 2, 2, KC, P], fp16, kind="ExternalInput").ap()
    wv16_d = nc.dram_tensor("wv16", [P, KC, D], fp16, kind="ExternalInput").ap()
    wo16_d = nc.dram_tensor("wo16", [P, KC, D], fp16, kind="ExternalInput").ap()
    vones_d = nc.dram_tensor("vones", [P, NKT, H, DH], fp16, kind="ExternalInput").ap()
    bo_d = nc.dram_tensor("bo", [D], fp32, kind="ExternalInput").ap()
    out_d = nc.dram_tensor("out", [NTOK_Q, D], fp16, kind="ExternalOutput").ap()
    if debug:
        dbg_qt = nc.dram_tensor("dbg_qt", [P, 2, 2, NTOK_Q], fp8, kind="ExternalOutput").ap()
        dbg_kt = nc.dram_tensor("dbg_kt", [P, 2, 2, NTOK_K], fp8, kind="ExternalOutput").ap()
        dbg_v = nc.dram_tensor("dbg_v", [P, NKT, H, P], fp16, kind="ExternalOutput").ap()
        dbg_ot = nc.dram_tensor("dbg_ot", [P, KC, NTOK_Q], fp16, kind="ExternalOutput").ap()

    with ExitStack() as ctx:
        tc = ctx.enter_context(tile.TileContext(nc))
        singles = ctx.enter_context(tc.tile_pool(name="singles", bufs=1))
        fpool = ctx.enter_context(tc.tile_pool(name="fpool", bufs=3))
        epool = ctx.enter_context(tc.tile_pool(name="epool", bufs=3))
        rpool = ctx.enter_context(tc.tile_pool(name="rpool", bufs=2))
        mmps = ctx.enter_context(tc.tile_pool(name="mmps", bufs=2, space="PSUM"))
        scps = ctx.enter_context(tc.tile_pool(name="scps", bufs=2, space="PSUM"))
        accps = ctx.enter_context(tc.tile_pool(name="accps", bufs=1, space="PSUM"))

        # ---- static inputs ----
        wq16 = singles.tile([P, 2, 2, KC, P], fp16)
        nc.sync.dma_start(out=wq16, in_=wq16_d)
        wk16 = singles.tile([P, 2, 2, KC, P], fp16)
        nc.sync.dma_start(out=wk16, in_=wk16_d)
        wv16 = singles.tile([P, KC, D], fp16)
        nc.sync.dma_start(out=wv16, in_=wv16_d)
        wo16 = singles.tile([P, KC, D], fp16)
        nc.sync.dma_start(out=wo16, in_=wo16_d)
        xk16 = singles.tile([P, KC, NTOK_K], fp16)
        nc.sync.dma_start(out=xk16, in_=xk16_d)
        xq16 = singles.tile([P, KC, NTOK_Q], fp16)
        nc.sync.dma_start(out=xq16, in_=xq16_d)
        bo_rep = singles.tile([P, D], fp32)
        bo_bcast = bass.AP(tensor=bo_d.tensor, offset=bo_d.offset,
                           ap=[[0, P], [1, D]])
        nc.gpsimd.dma_start(out=bo_rep, in_=bo_bcast)

        KT8 = singles.tile([P, 2, 2, NTOK_K], fp8, name="KT8")
        QT8 = singles.tile([P, 2, 2, NTOK_Q], fp8, name="QT8")
        V128 = singles.tile([P, NKT, H, P], fp16, name="V128")
        outT = singles.tile([P, KC, NTOK_Q], fp16, name="outT")

        # validity columns of V at 0:DH (denominator lands in PSUM rows
        # 0:64 because the custom-DVE reciprocal drops input partition
        # offsets); V dims at DH:P
        nc.sync.dma_start(out=V128[:, :, :, 0:DH], in_=vones_d)

        # ---- projection unit emitters ----
        def qk_proj(dst, w16, x16, qs, w):
            for jj in range(2):
                for ii in range(2):
                    ps = mmps.tile([P, 512], fp32, name="qk_ps", tag="mm")
                    for kc in range(KC):
                        nc.tensor.matmul(
                            ps[:, :w],
                            w16[:, jj, ii, kc, :],
                            x16[:, kc, qs : qs + w],
                            start=(kc == 0), stop=(kc == KC - 1),
                        )
                    nc.vector.tensor_copy(
                        out=dst[:, jj, ii, qs : qs + w], in_=ps[:, :w]
                    )

        def v_proj(kt):
            ps = mmps.tile([P, 512], fp32, name="v_ps", tag="mm")
            for kc in range(KC):
                nc.tensor.matmul(
                    ps,
                    xk16[:, kc, kt * P : (kt + 1) * P],
                    wv16[:, kc, :],
                    start=(kc == 0), stop=(kc == KC - 1),
                )
            nc.vector.tensor_copy(
                out=V128[:, kt, :, DH:P],
                in_=ps.rearrange("p (h d) -> p h d", h=H),
            )

        def o_proj(qt):
            ps = mmps.tile([P, 512], fp32, name="o_ps", tag="mm")
            for g in range(KC):
                nc.tensor.matmul(
                    ps,
                    outT[:, g, qt * P : (qt + 1) * P],
                    wo16[:, g, :],
                    start=(g == 0), stop=(g == KC - 1),
                )
            fout = fpool.tile([P, D], fp16, tag="fout")
            nc.vector.tensor_tensor(fout, ps, bo_rep, add)
            nc.sync.dma_start(out=out_d[qt * P : (qt + 1) * P, :], in_=fout)

        # ---- choreographed emission ----
        # PE must stay continuously busy (idle gaps drop it to the mid
        # p-state, halving throughput): attention batches software-pipeline
        # PVD one batch behind scores, and projection/output units drain
        # into the exp-wait gaps as fillers.
        fillers: list = []  # (tag, fn)
        emitted_tags = set()

        def fill(n):
            for _ in range(min(n, len(fillers))):
                tag, fn = fillers.pop(0)
                emitted_tags.add(tag)
                fn()

        def ensure_ready(tags):
            need = set(tags) - emitted_tags
            while need & {t for t, _ in fillers} or (
                need and any(t in need for t, _ in fillers)
            ):
                tag, fn = fillers.pop(0)
                emitted_tags.add(tag)
                fn()
                need -= {tag}

        # per-window K-proj / V-proj unit lists; Q-proj per chunk
        kq_done = set()

        def ensure_k(m):
            if ("k", m) in kq_done:
                return
            kq_done.add(("k", m))
            for qs in range(woff[m] * P, (woff[m] + windows[m]) * P, 512):
                qk_proj(KT8, wk16, xk16, qs,
                        min(512, (woff[m] + windows[m]) * P - qs))

        def ensure_v(m):
            if ("v", m) in kq_done:
                return
            kq_done.add(("v", m))
            for kt in range(woff[m], woff[m] + windows[m]):
                v_proj(kt)

        def ensure_q(qt_off, ntiles):
            if ("q", qt_off) in kq_done:
                return
            kq_done.add(("q", qt_off))
            qk_proj(QT8, wq16, xq16, qt_off * P, ntiles * P)

        # order chunks by window compute footprint so attention starts fast
        worder = sorted(range(len(windows)), key=lambda m: windows[m])
        corder = [ch for m in worder for ch in chunks if ch[2] == m]

        # queue later windows' K/V and later chunks' Q as fillers
        first = corder[0]
        ensure_k(first[2])
        ensure_v(first[2])
        ensure_q(first[0], first[1])
        for ch in corder[1:]:
            m = ch[2]
            if ("k", m) not in kq_done:
                kq_done.add(("k", m))
                for qs in range(woff[m] * P, (woff[m] + windows[m]) * P, 512):
                    w_ = min(512, (woff[m] + windows[m]) * P - qs)
                    fillers.append((("k", m),
                        lambda qs=qs, w_=w_: qk_proj(KT8, wk16, xk16, qs, w_)))
            if ("v", m) not in kq_done:
                kq_done.add(("v", m))
                for kt in range(woff[m], woff[m] + windows[m]):
                    fillers.append((("v", m), lambda kt=kt: v_proj(kt)))
            if ("q", ch[0]) not in kq_done:
                kq_done.add(("q", ch[0]))
                fillers.append((("q", ch[0]),
                    lambda qo=ch[0], nt=ch[1]: qk_proj(
                        QT8, wq16, xq16, qo * P, nt * P
                    )))

        pending_norm: list = []

        def emit_norm():
            while pending_norm:
                norm_fn, post = pending_norm.pop(0)
                norm_fn()
                fillers.extend((("o", id(p)), p) for p in post)

        def attn_chunk(qt_off, ntiles, m, cap):
            qs, w = qt_off * P, ntiles * P
            ktc = max(1, 512 // w)  # k-tiles per scores tile
            bs = 512 if w == 384 else w  # PSUM-bank-aligned block stride
            for hl in range(4):
                o_pd = accps.tile([P, 2, 512], fp32, name="o_pd", tag="o_pd")

                def scores_exp(kt0, kn):
                    s_t = scps.tile([P, 1024], fp32, name="s_t", tag="s_t")
                    e_t = epool.tile([P, 1024], fp16, name="e_t", tag="e_t")
                    for dk in range(kn):
                        ktg = woff[m] + kt0 + dk
                        for jj in range(2):
                            off = (dk * 2 + jj) * bs
                            nc.tensor.matmul(
                                s_t[:, off : off + w],
                                KT8[32 * hl : 32 * hl + 32, jj, :,
                                    ktg * P : (ktg + 1) * P],
                                QT8[32 * hl : 32 * hl + 32, jj, :, qs : qs + w],
                                start=True, stop=True, perf_mode=DR,
                                tile_position=(32 * hl, 0),
                            )
                    if bs == w:
                        nc.scalar.activation(
                            e_t[:, 0 : kn * 2 * w], s_t[:, 0 : kn * 2 * w],
                            Exp, scale=0.125,
                        )
                    else:
                        nc.scalar.activation(
                            e_t.rearrange("p (j q) -> p j q", j=2)[:, :, :w],
                            s_t.rearrange("p (j q) -> p j q", j=2)[:, :, :w],
                            Exp, scale=0.125,
                        )
                    return e_t

                def pvd(kt0, kn, e_t):
                    for dk in range(kn):
                        ktg = woff[m] + kt0 + dk
                        kt = kt0 + dk
                        for jj in range(2):
                            off = (dk * 2 + jj) * bs
                            nc.tensor.matmul(
                                o_pd[:, jj, :w],
                                V128[:, ktg, hl + 4 * jj, :],
                                e_t[:, off : off + w],
                                start=(kt == 0), stop=(kt == cap - 1),
                                skip_group_check=True,
                            )

                pend = None
                for kt0 in range(0, cap, ktc):
                    kn = min(ktc, cap - kt0)
                    e_t = scores_exp(kt0, kn)
                    if pend is None:
                        # batch 0 of this hl: place the previous normalize
                        # (DVE) plus a filler into the exp-wait gap
                        emit_norm()
                        fill(1)
                    else:
                        pvd(*pend)
                        fill(1)
                    pend = (kt0, kn, e_t)
                pvd(*pend)

                def norm(hl=hl, o_pd=o_pd):
                    rrep = rpool.tile([DH, 2, 512], fp32, tag="rrep")
                    nc.vector.reciprocal_approx_fast(
                        out=rrep[:, :, :w], in_=o_pd[0:DH, :, :w]
                    )
                    for jj in range(2):
                        nc.vector.tensor_tensor(
                            outT[DH * jj : DH * jj + DH, hl, qs : qs + w],
                            o_pd[DH:P, jj, :w],
                            rrep[:, jj, :w],
                            mult,
                        )

                post = []
                if hl == 3:
                    post = [
                        (lambda qt=qt: o_proj(qt))
                        for qt in range(qt_off, qt_off + ntiles)
                    ]
                pending_norm.append((norm, post))

        for ci, (qt_off, ntiles, m, cap) in enumerate(corder):
            ensure_ready([("k", m), ("v", m), ("q", qt_off)])
            attn_chunk(qt_off, ntiles, m, cap)
        emit_norm()
        while fillers:
            tag, fn = fillers.pop(0)
            fn()

        if debug:
            nc.sync.dma_start(out=dbg_qt, in_=QT8)
            nc.sync.dma_start(out=dbg_kt, in_=KT8)
            nc.sync.dma_start(out=dbg_v, in_=V128)
            nc.sync.dma_start(out=dbg_ot, in_=outT)

    nc.compile()
    return nc


def _get_program(windows, chunks, debug):
    key = (windows, chunks, debug)
    if key not in _BUILD_CACHE:
        _BUILD_CACHE[key] = _build_bass(windows, chunks, debug)
    return _BUILD_CACHE[key]


# --------------------------------------------------------------------------
# host glue
# --------------------------------------------------------------------------

def _xt(tokens_x):
    """[T, D] fp32 -> [P, KC, T] transposed layout."""
    t = tokens_x.T.reshape(KC, P, tokens_x.shape[0]).transpose(1, 0, 2)
    return np.ascontiguousarray(t)


def _first_qt(chunks, m):
    for (qt_off, ntiles, mm, cap) in chunks:
        if mm == m:
            return qt_off
    raise ValueError(m)


def kernel(x, seq_lens, Wq, Wk, Wv, Wo, bo) -> np.ndarray:
    from concourse.bass_utils import run_bass_kernel_spmd

    x = np.asarray(x, dtype=np.float32)
    seq_lens_np = np.asarray(seq_lens, dtype=np.int32)
    Wq = np.asarray(Wq, dtype=np.float32)
    Wk = np.asarray(Wk, dtype=np.float32)
    Wv = np.asarray(Wv, dtype=np.float32)
    Wo = np.asarray(Wo, dtype=np.float32)
    bo = np.asarray(bo, dtype=np.float32)

    windows, chunks, assign = _solve_structure(seq_lens_np)
    NKT = sum(windows)
    NQ = sum(c[1] for c in chunks)
    woff = [0]
    for w_ in windows:
        woff.append(woff[-1] + w_)

    debug = bool(int(os.environ.get("KERNEL_DEBUG", "0")))
    nc = _get_program(windows, chunks, debug)

    # weight pre-arrangement (shared across cores)
    pidx = np.arange(P)
    hl_of = pidx // 32
    dlow = pidx % 32
    col = np.zeros((2, 2, P), dtype=np.int64)
    for jj in range(2):
        for ii in range(2):
            col[jj, ii] = 64 * (hl_of + 4 * jj) + dlow + 32 * ii

    def arrange_qk(W):
        # [c(128), j, i, kc, m] = W[kc*128+c, col(m,j,i)]
        a = np.zeros((P, 2, 2, KC, P), dtype=np.float32)
        for kc in range(KC):
            rows = np.arange(P) + kc * 128
            for jj in range(2):
                for ii in range(2):
                    a[:, jj, ii, kc, :] = W[rows[:, None], col[jj, ii][None, :]]
        return a.astype(np.float16)

    wq16 = arrange_qk(Wq)
    wk16 = arrange_qk(Wk)
    wv16 = np.ascontiguousarray(
        Wv.reshape(KC, P, D).transpose(1, 0, 2)
    ).astype(np.float16)
    wo16 = np.zeros((P, KC, D), dtype=np.float32)
    for g in range(KC):
        rows = 64 * (g + 4 * (pidx // 64)) + pidx % 64
        wo16[:, g, :] = Wo[rows, :]
    wo16 = wo16.astype(np.float16)

    in_maps = []
    for c in range(N_CORES):
        xk = np.zeros((NKT * P, D), dtype=np.float32)
        vones = np.zeros((P, NKT, H, DH), dtype=np.float16)
        for m, a in enumerate(assign[c]):
            if a is None:
                continue
            seq = a[0]
            L = int(seq_lens_np[seq])
            nkt_par = min(math.ceil(L / P), windows[m])
            n = min(L, nkt_par * P)
            xk[woff[m] * P : woff[m] * P + n] = x[seq, :n]
            pos = (np.arange(windows[m])[None, :] * P
                   + np.arange(P)[:, None])
            valid = (pos < L).astype(np.float16)  # [P, win]
            vones[:, woff[m] : woff[m] + windows[m], :, :] = (
                valid[:, :, None, None]
            )
        xq = np.zeros((NQ * P, D), dtype=np.float32)
        for (qt_off, ntiles, m, cap) in chunks:
            a = assign[c][m]
            if a is None:
                continue
            seq, qt0, njobs = a
            L = int(seq_lens_np[seq])
            for idx in range(ntiles):
                gidx = qt_off + idx
                job = gidx - _first_qt(chunks, m)
                if job < njobs:
                    r0 = (qt0 + job) * P
                    n = max(0, min(L - r0, P))
                    if n > 0:
                        xq[gidx * P : gidx * P + n] = x[seq, r0 : r0 + n]
        in_maps.append({
            "xq16": _xt(xq).astype(np.float16),
            "xk16": _xt(xk).astype(np.float16),
            "wq16": wq16, "wk16": wk16, "wv16": wv16, "wo16": wo16,
            "vones": vones, "bo": bo,
        })

    trace = bool(int(os.environ.get("KERNEL_TRACE", "0")))
    res = run_bass_kernel_spmd(
        nc, in_maps, core_ids=list(range(N_CORES)), trace=trace
    )
    kernel.last_results = res

    out = np.zeros((B, S, D), dtype=np.float32)
    for c in range(N_CORES):
        o = res.results[c]["out"].astype(np.float32)
        for (qt_off, ntiles, m, cap) in chunks:
            a = assign[c][m]
            if a is None:
                continue
            seq, qt0, njobs = a
            L = int(seq_lens_np[seq])
            for idx in range(ntiles):
                gidx = qt_off + idx
                job = gidx - _first_qt(chunks, m)
                if job < njobs:
                    r0 = (qt0 + job) * P
                    n = max(0, min(L - r0, P))
                    if n > 0:
                        out[seq, r0 : r0 + n] = o[gidx * P : gidx * P + n]
    return out
